# revision 1
# baseline (speedup 1.0000x reference)
"""Builder + host glue for the ViT attention kernel on 8 trn2 cores.

Reference computation (per batch b):
    qkv = x @ w_qkv.T ; q,k,v split; per head: softmax(q k^T / sqrt(dh)) v
    out = attn @ w_out.T + b_out

Sharding: data-parallel over batch (8 batches per core).
"""

import numpy as np
import ml_dtypes

import concourse.bass as bass
import concourse.tile as tile
from concourse import bacc, mybir
from concourse.bass_utils import run_bass_kernel_spmd

P = 128
B, N, D = 64, 197, 768
H, DH = 12, 64
NCORES = 8
BPC = B // NCORES          # 8 batches per core
T = BPC * N                # 1576 tokens per core
KT = D // P                # 6 contraction tiles
NPAIR = H // 2             # 6 head pairs
SCALE = DH ** -0.5
VW = (DH + 1) * H          # 780: v columns incl per-head ones column
N2 = 2 * N                 # 394
JT1 = N - P                # 69: second j-tile size

BF = mybir.dt.bfloat16
F32 = mybir.dt.float32
EXP = mybir.ActivationFunctionType.Exp

T_CHUNKS = [(0, 394), (394, 394), (788, 394), (1182, 394)]


def build_nc():
    nc = bacc.Bacc(
        "TRN2", target_bir_lowering=False, debug=False, num_devices=NCORES
    )
    xT = nc.dram_tensor("xT", [D, T], BF, kind="ExternalInput").ap()
    wqkT = nc.dram_tensor("wqkT", [D, 2 * D], BF, kind="ExternalInput").ap()
    wvT = nc.dram_tensor("wvT", [D, D], BF, kind="ExternalInput").ap()
    woT = nc.dram_tensor("woT", [D, D], BF, kind="ExternalInput").ap()
    bias = nc.dram_tensor("bias", [P, KT], F32, kind="ExternalInput").ap()
    outT = nc.dram_tensor("outT", [D, T], F32, kind="ExternalOutput").ap()

    with tile.TileContext(nc) as tc:
        with (
            tc.tile_pool(name="big", bufs=1) as big,
            tc.tile_pool(name="exp", bufs=12) as sb_exp,
            tc.tile_pool(name="rec", bufs=8) as sb_rec,
            tc.tile_pool(name="bsb", bufs=8) as sb_bsb,
            tc.tile_pool(name="osb", bufs=3) as sb_osb,
            tc.tile_pool(name="ps_pj", bufs=2, space="PSUM") as ps_pj,
            tc.tile_pool(name="ps_sc", bufs=3, space="PSUM") as ps_sc,
            tc.tile_pool(name="ps_o", bufs=3, space="PSUM") as ps_o,
        ):
            # ---- persistent buffers + input DMAs -------------------------
            ones1 = big.tile([1, DH], F32, tag="ones1")
            nc.vector.memset(ones1[:], 1.0)
            bias_sb = big.tile([P, KT], F32, tag="bias")
            nc.sync.dma_start(bias_sb[:], bias)

            # x: per (k, 394-chunk) tiles; wqk: per (k, 256-col group) tiles
            # so dependency tracking is chunk-granular (ramp overlap).
            x_sb = [
                [big.tile([P, 394], BF, tag=f"x{k}_{c}", name=f"x{k}_{c}") for c in range(4)]
                for k in range(KT)
            ]
            wqk_sb = [
                [big.tile([P, 256], BF, tag=f"wqk{k}_{g}", name=f"wqk{k}_{g}") for g in range(NPAIR)]
                for k in range(KT)
            ]  # granular tiles kept; DMA below fills pairs of groups per issue
            wv_sb = []
            wo_sb = []
            for k in range(KT):
                wv_sb.append(big.tile([P, D], BF, tag=f"wv{k}", name=f"wv{k}"))
                wo_sb.append(big.tile([P, D], BF, tag=f"wo{k}", name=f"wo{k}"))
            def dma_x(k, c, eng):
                t0, tl = T_CHUNKS[c]
                eng.dma_start(x_sb[k][c][:], xT[k * P : (k + 1) * P, t0 : t0 + tl])

            def dma_wqk(k, g, eng):
                c0 = g * 256
                eng.dma_start(
                    wqk_sb[k][g][:], wqkT[k * P : (k + 1) * P, c0 : c0 + 256]
                )

            # critical first QK group: x chunk0 on scalar, wqk group0 on sync
            for k in range(KT):
                dma_x(k, 0, nc.scalar)
                dma_wqk(k, 0, nc.sync)
            for c in range(1, 4):
                for k in range(KT):
                    dma_x(k, c, nc.scalar)
            for g in range(1, NPAIR):
                for k in range(KT):
                    dma_wqk(k, g, nc.sync)

            for k in range(KT):
                nc.sync.dma_start(wv_sb[k][:], wvT[k * P : (k + 1) * P, :])
            for k in range(KT):
                nc.gpsimd.dma_start(wo_sb[k][:], woT[k * P : (k + 1) * P, :])

            def x_ap(k, t0, tl):
                c = t0 // 394
                o = t0 - c * 394
                return x_sb[k][c][:, o : o + tl]

            def wqk_ap(k, m):
                # m in 0..11 over 1536 cols; group = 256 cols
                g, o = divmod(m * P, 256)
                return wqk_sb[k][g][:, o : o + P]

            # qk_sb[m]: m<6 -> q head-pair m ; m>=6 -> k head-pair m-6.
            # layout [e within pair (2 heads x 64), t global]
            qk_sb = [big.tile([P, T], BF, tag=f"qk{m}", name=f"qk{m}") for m in range(2 * NPAIR)]
            # v tiles per (batch, j-tile): [j, 12*(64+1)] with ones columns
            v_sb = [big.tile([P, VW], BF, tag=f"v{i}", name=f"v{i}") for i in range(2 * BPC)]
            for i in range(2 * BPC):
                ones_cols = v_sb[i][:].rearrange("p (h c) -> p h c", c=DH + 1)[
                    :, :, DH : DH + 1
                ]
                nc.gpsimd.memset(ones_cols, 1.0)
            # attention output, [e, t] layout, tiles per (pair, batch-pair)
            at_sb = [
                [big.tile([P, N2], BF, tag=f"at{p}_{b2}", name=f"at{p}_{b2}") for b2 in range(BPC // 2)]
                for p in range(NPAIR)
            ]

            # ---- QK projection: qkT[e, t] = (w_qk x^T) ------------------
            qk_alt = [0]
            for p in range(NPAIR):
                for m in (p, NPAIR + p):
                    for t0, tl in T_CHUNKS:
                        qk_alt[0] += 1
                        if qk_alt[0] % 5 < 3:
                            psum = ps_sc.tile([P, N2], F32, tag="sc", name="qksc")[:, :tl]
                        else:
                            psum = ps_pj.tile([P, 512], F32, tag="pj", name="pj")[:, :tl]
                        for k in range(KT):
                            nc.tensor.matmul(
                                psum,
                                wqk_ap(k, m),
                                x_ap(k, t0, tl),
                                start=(k == 0),
                                stop=(k == KT - 1),
                            )
                        nc.vector.tensor_copy(
                            out=qk_sb[m][:, t0 : t0 + tl], in_=psum
                        )

            # ---- V projection units (filler-interleaved) ----------------
            def vproj_unit(b, jt, c0, cl):
                def emit():
                    r0 = b * N + jt * P
                    rl = P if jt == 0 else JT1
                    i = 2 * b + jt
                    psum = ps_pj.tile([P, 512], F32, tag="pj", name="pjv")[:rl, :cl]
                    for k in range(KT):
                        nc.tensor.matmul(
                            psum,
                            x_ap(k, r0, rl),
                            wv_sb[k][:, c0 : c0 + cl],
                            start=(k == 0),
                            stop=(k == KT - 1),
                        )
                    hs = c0 // DH
                    nh = cl // DH
                    out_ap = v_sb[i][
                        :rl, (DH + 1) * hs : (DH + 1) * (hs + nh)
                    ].rearrange("p (h c) -> p h c", c=DH + 1)[:, :, 0:DH]
                    nc.scalar.copy(
                        out=out_ap,
                        in_=psum.rearrange("p (h c) -> p h c", c=DH),
                    )

                return emit

            def vproj_units(b):
                return [
                    vproj_unit(b, jt, c0, cl)
                    for jt in range(2)
                    for c0, cl in ((0, 512), (512, 256))
                ]

            # ---- out-projection units -----------------------------------
            op_alt = [0]

            def outproj_unit(b2, m):
                def emit():
                    t0 = b2 * N2
                    op_alt[0] += 1
                    if op_alt[0] % 2 == 0:
                        psum = ps_sc.tile([P, N2], F32, tag="sc", name="opsc")[:, :N2]
                    else:
                        psum = ps_pj.tile([P, 512], F32, tag="pj", name="pjo")[:, :N2]
                    for k in range(KT):
                        nc.tensor.matmul(
                            psum,
                            wo_sb[k][:, m * P : (m + 1) * P],
                            at_sb[k][b2][:],
                            start=(k == 0),
                            stop=(k == KT - 1),
                        )
                    osb = sb_osb.tile([P, 512], F32, tag="osb", name="osb")[:, :N2]
                    nc.scalar.activation(
                        osb,
                        psum,
                        mybir.ActivationFunctionType.Identity,
                        bias=bias_sb[:, m : m + 1],
                    )
                    nc.sync.dma_start(outT[m * P : (m + 1) * P, t0 : t0 + N2], osb)

                return emit

            # ---- one attention head-pair --------------------------------
            def emit_pair(b, p):
                tb = b * N
                qT = qk_sb[p]
                kTt = qk_sb[NPAIR + p]
                expT = []
                for h in (0, 1):
                    e0 = 64 * h
                    ps_s = ps_sc.tile([P, N2], F32, tag="sc", name="sc")
                    nc.tensor.matmul(
                        ps_s[0:P, 0:N],
                        kTt[e0 : e0 + DH, tb : tb + P],
                        qT[e0 : e0 + DH, tb : tb + N],
                        start=True,
                        stop=True,
                        tile_position=(e0, 0),
                    )
                    nc.tensor.matmul(
                        ps_s[0:JT1, N:N2],
                        kTt[e0 : e0 + DH, tb + P : tb + N],
                        qT[e0 : e0 + DH, tb : tb + N],
                        start=True,
                        stop=True,
                        tile_position=(e0, 0),
                    )
                    e = sb_exp.tile([P, N2], BF, tag="expT", name="expT")
                    nc.scalar.activation(e[:], ps_s[:], EXP)
                    expT.append(e)
                pso = ps_o.tile([DH + 1, N2], F32, tag="o", name="o")
                for h in (0, 1):
                    g = 2 * p + h
                    vc = (DH + 1) * g
                    nc.tensor.matmul(
                        pso[:, N * h : N * h + N],
                        v_sb[2 * b][0:P, vc : vc + DH + 1],
                        expT[h][0:P, 0:N],
                        start=True,
                        stop=False,
                    )
                    nc.tensor.matmul(
                        pso[:, N * h : N * h + N],
                        v_sb[2 * b + 1][0:JT1, vc : vc + DH + 1],
                        expT[h][0:JT1, N:N2],
                        start=False,
                        stop=True,
                    )
                # S row -> SBUF (base 0: custom DVE/GpSimd ops require it),
                # approx reciprocal, GpSimd partition broadcast, normalize
                # straight out of PSUM (single PSUM operand per DVE op)
                s_sb = sb_rec.tile([1, N2], F32, tag="s_sb", name="s_sb")
                nc.vector.tensor_copy(out=s_sb[:], in_=pso[DH : DH + 1, :])
                rec = sb_rec.tile([1, N2], F32, tag="rec", name="rec")
                nc.vector.reciprocal_approx_fast(out=rec[:], in_=s_sb[:])
                bsb = sb_bsb.tile([DH, N2], F32, tag="bsb", name="bsb")
                nc.gpsimd.partition_broadcast(bsb[:], rec[:])
                for h in (0, 1):
                    nc.vector.tensor_mul(
                        out=at_sb[p][b // 2][
                            64 * h : 64 * h + DH, N * (b % 2) : N * (b % 2) + N
                        ],
                        in0=pso[0:DH, N * h : N * h + N],
                        in1=bsb[:, N * h : N * h + N],
                    )

            # ---- driver: attention with 1:1 projection filler -----------
            from collections import deque

            filler = deque()  # items: (kind, batch, emit_fn)
            for u in vproj_units(0) + vproj_units(1):
                u()
            filler.extend(("v", 2, u) for u in vproj_units(2))
            for b in range(BPC):
                # v tiles for batch b must be traced before its pairs
                for item in [it for it in filler if it[0] == "v" and it[1] <= b]:
                    filler.remove(item)
                    item[2]()
                for p in range(NPAIR):
                    emit_pair(b, p)
                    if filler:
                        filler.popleft()[2]()
                if b + 3 < BPC:
                    filler.extend(("v", b + 3, u) for u in vproj_units(b + 3))
                if b % 2 == 1:
                    filler.extend(
                        ("o", b, outproj_unit(b // 2, m)) for m in range(KT)
                    )
            while filler:
                filler.popleft()[2]()

    nc.compile()
    return nc


def host_in_maps(x, w_qkv, w_out, b_out):
    """Full fp32 inputs -> list of 8 per-core input dicts (bf16)."""
    bf16 = ml_dtypes.bfloat16
    wq = w_qkv[0:D] * SCALE
    wk = w_qkv[D : 2 * D]
    wv = w_qkv[2 * D : 3 * D]
    wqkT = np.ascontiguousarray(np.concatenate([wq, wk], axis=0).T).astype(bf16)
    wvT = np.ascontiguousarray(wv.T).astype(bf16)
    woT = np.ascontiguousarray(w_out.T).astype(bf16)
    bias = np.ascontiguousarray(b_out.reshape(KT, P).T).astype(np.float32)
    in_maps = []
    for c in range(NCORES):
        xc = x[c * BPC : (c + 1) * BPC].reshape(T, D)
        xT = np.ascontiguousarray(xc.T).astype(bf16)
        in_maps.append(
            {"xT": xT, "wqkT": wqkT, "wvT": wvT, "woT": woT, "bias": bias}
        )
    return in_maps


def host_gather(results):
    """8 per-core {outT: [768, 1576] fp32} -> full [64, 197, 768] fp32."""
    out = np.empty((B, N, D), dtype=np.float32)
    for c in range(NCORES):
        oc = results[c]["outT"]  # [D, T]
        out[c * BPC : (c + 1) * BPC] = oc.T.reshape(BPC, N, D)
    return out



_NC_CACHE = []


def kernel(x, w_qkv, w_out, b_out):
    """Full-input entry point: shards batch over 8 NeuronCores, runs the
    Bass kernel, gathers the full [64, 197, 768] fp32 output."""
    if not _NC_CACHE:
        _NC_CACHE.append(build_nc())
    nc = _NC_CACHE[0]
    in_maps = host_in_maps(
        np.asarray(x, dtype=np.float32),
        np.asarray(w_qkv, dtype=np.float32),
        np.asarray(w_out, dtype=np.float32),
        np.asarray(b_out, dtype=np.float32),
    )
    res = run_bass_kernel_spmd(nc, in_maps, core_ids=list(range(NCORES)))
    return host_gather(res.results)

